# revision 25
# baseline (speedup 1.0000x reference)
"""Trainium2 Bass kernel for nn_BeliefModuleOld (segment_reduce).

Reference semantics per batch element b and treat type tt:
  valid[t] = (vision[b,t] != 0) and (max(visible_treats[b,t,tt,0:5]) > 0.5)
  out[b,tt,:] = visible_treats[b, last valid t, tt, :]  (or [0,0,0,0,0,1] if none)

Strategy: pure data-parallel over batch, 8 cores, BC = 125,000 elems/core.

Layout: 128 SBUF partitions (a multiple of 16 — the DMA descriptor
round-robin then lines up with the fixed partition->SBUF-port map; a
125-partition layout measures ~164 GB/s/core vs ~414-640 GB/s/core for
128), q batch elements per partition per tile. Main loop covers
128*q*nmain elements; a 72-element tail rides partitions 0..71 once per
pass. Per main tile:
  - x [128, q,5,2,6] f32 on the sync HWDGE ring; v [128, q,5] i32 and the
    o store on the scalar HWDGE ring (concurrent rings with independent
    transfers don't interfere; splitting ONE transfer across rings does)
  - hm[t,tt] = max over d<5 of x (tensor_max tree on DVE)
  - valid = (hm > 0.5) * vision  (scalar_tensor_tensor)
  - out initialized to the default vector via ACT copy from a const tile,
    then for t=0..4 ascending copy_predicated(out, valid[t], x[t]) --
    last valid wins
  - o [128, q,2,6] stored back
"""

import numpy as np

import concourse.bass as bass
import concourse.bacc as bacc
import concourse.tile as tile
from concourse import mybir
from concourse.alu_op_type import AluOpType
from concourse.bass_utils import run_bass_kernel_spmd

B, T, NT, D = 1_000_000, 5, 2, 6
NCORES = 8
BC = B // NCORES  # 125,000 per core
P = 128
Q = 244           # batch elements per partition per main tile
ROW = T * NT * D  # 60 floats per batch element


def _copy_predicated(eng, out, mask, data):
    # Same as BassVectorEngine.copy_predicated but with opt=False lowering so
    # the three operand APs keep identical [p, f, nt, d] structure (the
    # broadcast mask AP cannot merge dims; unoptimized APs keep the sim's
    # np.where shapes aligned and the HW element streams in lockstep).
    return eng.add_instruction(
        mybir.InstCopyPredicated(
            name=f"I-{eng.bass.next_id()}",
            ins=[eng.lower_ap(mask, opt=False), eng.lower_ap(data, opt=False)],
            outs=[eng.lower_ap(out, opt=False)],
        )
    )


def _chain(nc, wpool, xs, vs, os_, pp, qq):
    """The per-chunk compute chain. APs: xs [pp, qq, T, NT, D];
    vs [pp, qq, T]; os_ [pp, qq, NT, D] (pre-initialized to default)."""
    fdt = mybir.dt.float32
    a = wpool.tile([pp, qq, T, NT], fdt, tag=f"a{qq}")
    bt = wpool.tile([pp, qq, T, NT], fdt, tag=f"b{qq}")
    valid = wpool.tile([pp, qq, T, NT], mybir.dt.uint8, tag=f"m{qq}")
    nc.vector.tensor_max(a[:], xs[:, :, :, :, 0], xs[:, :, :, :, 1])
    nc.vector.tensor_max(bt[:], xs[:, :, :, :, 2], xs[:, :, :, :, 3])
    nc.vector.tensor_max(a[:], a[:], bt[:])
    nc.vector.tensor_max(a[:], a[:], xs[:, :, :, :, 4])
    vb = vs.unsqueeze(3).broadcast_to((pp, qq, T, NT))
    nc.vector.scalar_tensor_tensor(
        out=valid[:], in0=a[:], scalar=0.5, in1=vb,
        op0=AluOpType.is_gt, op1=AluOpType.mult,
    )
    for t in range(T):
        mask = valid[:, :, t, :].unsqueeze(3).broadcast_to((pp, qq, NT, D))
        _copy_predicated(nc.vector, os_, mask, xs[:, :, t, :, :])


def _build_pmajor(
    nc, x, v, o, reps, q, nmain, nb, tail, sgroup, csplit, te, mode, ring1=1,
    nov=0, nos=0, notail=0, schunk=0, obf=0,
):
    """Partition-major layout: partition pp owns batch elems
    [pp*nq, (pp+1)*nq), nq = nmain*q. Tile i covers q of them.

    ring1=1: ALL DMA rides the sync HWDGE ring as one FIFO — concurrent
    rings make the 16 SDMA engines round-robin between queues, which
    measures ~2-4x slower than a single queue (x-only 690 GB/s; +stores
    on a second ring -> 43us of added wall for 6MB). FIFO order per pass:
      V, L0..L3, TL(tail loads), S0..S1(group stores), TS(tail store)
    Stores are deferred to the end so their compute dependencies are
    long-satisfied when the ring reaches them (no head-of-line stall).
    vision is double-buffered so the next pass's preload doesn't WAR-wait
    on this pass's compute."""
    p = P
    fdt = mybir.dt.float32
    nq = nmain * q
    assert nmain % sgroup == 0
    ngroups = nmain // sgroup
    qc = q // csplit
    assert qc * csplit == q

    xr = x[0:nb].rearrange("(p n q) t nt d -> n p (q t nt d)", p=p, n=nmain)
    vr = v[0:nb].rearrange("(p nq) t -> p (nq t)", p=p)
    org = o[0:nb].rearrange("(p g h) nt d -> g p (h nt d)", p=p, g=ngroups)
    emap = {"s": nc.sync, "a": nc.scalar, "g": nc.gpsimd}
    # ring1: 1 = everything on sync; 0 = v+o on scalar; 2 = v sync, o gpsimd;
    # 3 = v sync, o scalar
    tld = emap[te] if ring1 == 0 else nc.sync
    vld = nc.scalar if ring1 == 0 else nc.sync
    ost = {0: nc.scalar, 1: nc.sync, 2: nc.gpsimd, 3: nc.scalar}[ring1]

    with tile.TileContext(nc) as tc:
        with (
            tc.tile_pool(name="xs", bufs=2) as xpool,
            tc.tile_pool(name="vs", bufs=1) as vpool,
            tc.tile_pool(name="os", bufs=2) as opool,
            tc.tile_pool(name="wk", bufs=2) as wpool,
        ):
            dflt = wpool.tile([p, NT, D], fdt, tag="dflt", bufs=1)
            nc.gpsimd.memset(dflt[:, :, 0:5], 0.0)
            nc.gpsimd.memset(dflt[:, :, 5:6], 1.0)

            vt = None
            pending = []  # deferred (ap_out, tile) stores
            for it in range(reps * nmain):
                i = it % nmain
                g, j = i // sgroup, i % sgroup
                if i == 0 and not nov:
                    # once per pass: the whole vision tensor in one DMA
                    vt = vpool.tile([p, nq, T], mybir.dt.int32, tag="v", bufs=1)
                    vld.dma_start(
                        out=vt[:].rearrange("p nq t -> p (nq t)"), in_=vr
                    )
                xt = xpool.tile([p, q, T, NT, D], fdt, tag="x")
                nc.sync.dma_start(
                    out=xt[:].rearrange("p q t nt d -> p (q t nt d)"), in_=xr[i]
                )
                if mode == "dma":
                    if j == sgroup - 1:
                        pending.append(
                            (
                                org[g],
                                xt[:].rearrange("p q t nt d -> p (q t nt d)")[
                                    :, 0 : sgroup * q * NT * D
                                ],
                            )
                        )
                elif obf:
                    # stage each chunk in a small f32 tile, cast into the
                    # group's bf16 tile (halves store bytes AND SBUF)
                    if j == 0:
                        ot16 = opool.tile(
                            [p, sgroup * q, NT, D], mybir.dt.bfloat16, tag="o16"
                        )
                    for c in range(csplit):
                        sl = slice(c * qc, (c + 1) * qc)
                        vsl = slice(i * q + c * qc, i * q + (c + 1) * qc)
                        otc = opool.tile([p, qc, NT, D], fdt, tag="oc")
                        nc.gpsimd.memset(otc[:, :, :, 0:5], 0.0)
                        nc.gpsimd.memset(otc[:, :, :, 5:6], 1.0)
                        _chain(nc, wpool, xt[:, sl], vt[:, vsl], otc[:], p, qc)
                        nc.scalar.copy(
                            ot16[:, j * q + c * qc : j * q + (c + 1) * qc],
                            otc[:],
                        )
                    if j == sgroup - 1:
                        pending.append(
                            (org[g], ot16[:].rearrange("p h nt d -> p (h nt d)"))
                        )
                else:
                    if j == 0:
                        ot = opool.tile([p, sgroup * q, NT, D], fdt, tag="o")
                        nc.scalar.copy(
                            ot[:],
                            dflt[:]
                            .unsqueeze(1)
                            .broadcast_to((p, sgroup * q, NT, D)),
                        )
                    for c in range(csplit):
                        sl = slice(c * qc, (c + 1) * qc)
                        vsl = slice(i * q + c * qc, i * q + (c + 1) * qc)
                        _chain(
                            nc, wpool, xt[:, sl], vt[:, vsl],
                            ot[:, j * q + c * qc : j * q + (c + 1) * qc], p, qc,
                        )
                    if j == sgroup - 1:
                        pending.append(
                            (org[g], ot[:].rearrange("p h nt d -> p (h nt d)"))
                        )

                if i == nmain - 1:
                    if nos:
                        pending = []
                    ot2 = None
                    if tail and not notail:
                        xt2 = xpool.tile([tail, 1, T, NT, D], fdt, tag="xt")
                        tld.dma_start(
                            out=xt2[:].rearrange("p q t nt d -> p (q t nt d)"),
                            in_=x[nb:BC].rearrange(
                                "(p q) t nt d -> p (q t nt d)", q=1
                            ),
                        )
                        vt2 = vpool.tile(
                            [tail, 1, T], mybir.dt.int32, tag="vt", bufs=2
                        )
                        tld.dma_start(
                            out=vt2[:].rearrange("p q t -> p (q t)"),
                            in_=v[nb:BC].rearrange("(p q) t -> p (q t)", q=1),
                        )
                        ot2 = opool.tile([tail, 1, NT, D], fdt, tag="ot")
                        nc.scalar.copy(
                            ot2[:],
                            dflt[0:tail]
                            .unsqueeze(1)
                            .broadcast_to((tail, 1, NT, D)),
                        )
                        if mode != "dma":
                            _chain(nc, wpool, xt2[:], vt2[:], ot2[:], tail, 1)
                        if obf:
                            ot2_16 = opool.tile(
                                [tail, 1, NT, D], mybir.dt.bfloat16, tag="ot16"
                            )
                            nc.scalar.copy(ot2_16[:], ot2[:])
                            ot2 = ot2_16
                    sch = schunk if schunk else sgroup * q * NT * D
                    for out_ap, in_ap in pending:
                        nsc = (sgroup * q * NT * D) // sch
                        for k in range(nsc):
                            ost.dma_start(
                                out=out_ap[:, k * sch : (k + 1) * sch],
                                in_=in_ap[:, k * sch : (k + 1) * sch],
                            )
                    pending = []
                    if ot2 is not None:
                        tld.dma_start(
                            out=o[nb:BC].rearrange(
                                "(p q) nt d -> p (q nt d)", q=1
                            ),
                            in_=ot2[:].rearrange("p q nt d -> p (q nt d)"),
                        )

    nc.compile()
    return nc


def build_nc(
    reps=1,
    q=Q,
    xe="s",
    ve="s",
    oe="a",
    tail_every_rep=True,
    mode="full",
    csplit=2,
    layout="p",
    sgroup=2,
    te="a",
    ring1=0,
    nov=0,
    nos=0,
    notail=0,
    schunk=0,
    obf=0,
):
    p = P
    nmain = BC // (p * q)
    nb = p * q * nmain
    tail = BC - nb
    fdt = mybir.dt.float32

    nc = bacc.Bacc("TRN2", target_bir_lowering=False)
    x = nc.dram_tensor("x", [BC, T, NT, D], fdt, kind="ExternalInput")
    v = nc.dram_tensor("v", [BC, T], mybir.dt.int32, kind="ExternalInput")
    odt = mybir.dt.bfloat16 if obf else fdt
    o = nc.dram_tensor("o", [BC, NT, D], odt, kind="ExternalOutput")

    if layout == "p":
        return _build_pmajor(
            nc, x, v, o, reps, q, nmain, nb, tail, sgroup, csplit, te, mode, ring1,
            nov, nos, notail, schunk, obf,
        )
    assert not obf, "obf only implemented for layout='p'" 

    xr = x[0:nb].rearrange("(n p q) t nt d -> n p (q t nt d)", p=p, q=q)
    vr = v[0:nb].rearrange("(n p q) t -> n p (q t)", p=p, q=q)
    orr = o[0:nb].rearrange("(n p q) nt d -> n p (q nt d)", p=p, q=q)

    engs = {}

    def compute(xs, vs, os_, pp, qq):
        # APs: xs [pp, qq, T, NT, D]; vs [pp, qq, T]; os_ [pp, qq, NT, D]
        a = engs["wpool"].tile([pp, qq, T, NT], fdt, tag=f"a{qq}")
        bt = engs["wpool"].tile([pp, qq, T, NT], fdt, tag=f"b{qq}")
        valid = engs["wpool"].tile([pp, qq, T, NT], mybir.dt.uint8, tag=f"m{qq}")
        nc.vector.tensor_max(a[:], xs[:, :, :, :, 0], xs[:, :, :, :, 1])
        nc.vector.tensor_max(bt[:], xs[:, :, :, :, 2], xs[:, :, :, :, 3])
        nc.vector.tensor_max(a[:], a[:], bt[:])
        nc.vector.tensor_max(a[:], a[:], xs[:, :, :, :, 4])
        vb = vs.unsqueeze(3).broadcast_to((pp, qq, T, NT))
        nc.vector.scalar_tensor_tensor(
            out=valid[:], in0=a[:], scalar=0.5, in1=vb,
            op0=AluOpType.is_gt, op1=AluOpType.mult,
        )
        for t in range(T):
            mask = valid[:, :, t, :].unsqueeze(3).broadcast_to((pp, qq, NT, D))
            _copy_predicated(nc.vector, os_, mask, xs[:, :, t, :, :])

    with tile.TileContext(nc) as tc:
        with (
            tc.tile_pool(name="xs", bufs=2) as xpool,
            tc.tile_pool(name="vs", bufs=2) as vpool,
            tc.tile_pool(name="os", bufs=2) as opool,
            tc.tile_pool(name="wk", bufs=2) as wpool,
        ):
            engs["wpool"] = wpool
            emap = {"s": nc.sync, "a": nc.scalar, "g": nc.gpsimd}
            xld, vld, ost = emap[xe], emap[ve], emap[oe]

            dflt = wpool.tile([p, NT, D], fdt, tag="dflt", bufs=1)
            nc.gpsimd.memset(dflt[:, :, 0:5], 0.0)
            nc.gpsimd.memset(dflt[:, :, 5:6], 1.0)

            if mode == "compute":
                # bench: one resident tile, compute chain repeated
                xt = xpool.tile([p, q, T, NT, D], fdt, tag="x")
                xld.dma_start(
                    out=xt[:].rearrange("p q t nt d -> p (q t nt d)"), in_=xr[0]
                )
                vt = vpool.tile([p, q, T], mybir.dt.int32, tag="v")
                vld.dma_start(
                    out=vt[:].rearrange("p q t -> p (q t)"), in_=vr[0]
                )
                for it in range(reps * nmain):
                    ot = opool.tile([p, q, NT, D], fdt, tag="o")
                    nc.scalar.copy(
                        ot[:], dflt[:].unsqueeze(1).broadcast_to((p, q, NT, D))
                    )
                    compute(xt[:], vt[:], ot[:], p, q)
                ost.dma_start(
                    out=orr[0], in_=ot[:].rearrange("p q nt d -> p (q nt d)")
                )
                nc.compile()
                return nc

            qc = q // csplit
            assert qc * csplit == q
            for it in range(reps * nmain):
                i = it % nmain
                xt = xpool.tile([p, q, T, NT, D], fdt, tag="x")
                nc_x = xt[:].rearrange("p q t nt d -> p (q t nt d)")
                xld.dma_start(out=nc_x, in_=xr[i])
                vt = vpool.tile([p, q, T], mybir.dt.int32, tag="v")
                vld.dma_start(
                    out=vt[:].rearrange("p q t -> p (q t)"), in_=vr[i]
                )
                if mode == "dma":
                    ost.dma_start(
                        out=orr[i],
                        in_=xt[:].rearrange("p q t nt d -> p (q t nt d)")[
                            :, 0 : q * NT * D
                        ],
                    )
                    continue
                ot = opool.tile([p, q, NT, D], fdt, tag="o")
                nc.scalar.copy(
                    ot[:], dflt[:].unsqueeze(1).broadcast_to((p, q, NT, D))
                )
                for c in range(csplit):
                    sl = slice(c * qc, (c + 1) * qc)
                    compute(xt[:, sl], vt[:, sl], ot[:, sl], p, qc)
                ost.dma_start(
                    out=orr[i], in_=ot[:].rearrange("p q nt d -> p (q nt d)")
                )

                if tail and i == nmain - 1 and (tail_every_rep or it == reps * nmain - 1):
                    # tail: one batch element per partition on partitions 0..tail-1
                    xt2 = xpool.tile([tail, 1, T, NT, D], fdt, tag="xt")
                    xld.dma_start(
                        out=xt2[:].rearrange("p q t nt d -> p (q t nt d)"),
                        in_=x[nb:BC].rearrange("(p q) t nt d -> p (q t nt d)", q=1),
                    )
                    vt2 = vpool.tile([tail, 1, T], mybir.dt.int32, tag="vt")
                    vld.dma_start(
                        out=vt2[:].rearrange("p q t -> p (q t)"),
                        in_=v[nb:BC].rearrange("(p q) t -> p (q t)", q=1),
                    )
                    ot2 = opool.tile([tail, 1, NT, D], fdt, tag="ot")
                    nc.scalar.copy(
                        ot2[:],
                        dflt[0:tail].unsqueeze(1).broadcast_to((tail, 1, NT, D)),
                    )
                    compute(xt2[:], vt2[:], ot2[:], tail, 1)
                    ost.dma_start(
                        out=o[nb:BC].rearrange("(p q) nt d -> p (q nt d)", q=1),
                        in_=ot2[:].rearrange("p q nt d -> p (q nt d)"),
                    )

    nc.compile()
    return nc


_NC = None


def run_spmd(visible_treats: np.ndarray, vision: np.ndarray, **kwargs):
    global _NC
    if _NC is None:
        _NC = build_nc()
    if not kwargs.get("trace"):
        # NTFF profiling needs antenv.axon_hooks, absent in this container; a
        # stray BASS_TRACE env var would otherwise crash the run.
        import os

        os.environ.setdefault("BASS_NEVER_TRACE", "1")
    vt = np.ascontiguousarray(visible_treats, dtype=np.float32)
    vi = np.ascontiguousarray(vision, dtype=np.int32)
    in_maps = [
        {
            "x": vt[c * BC : (c + 1) * BC],
            "v": vi[c * BC : (c + 1) * BC],
        }
        for c in range(NCORES)
    ]
    return run_bass_kernel_spmd(_NC, in_maps, core_ids=list(range(NCORES)), **kwargs)


def kernel(visible_treats: np.ndarray, vision: np.ndarray) -> np.ndarray:
    res = run_spmd(visible_treats, vision)
    return np.concatenate(
        [np.asarray(r["o"]).astype(np.float32) for r in res.results], axis=0
    )
